# revision 3
# baseline (speedup 1.0000x reference)
"""Trainium2 Bass kernel for nn_LocalRNN: 8-step CTRNN over sliding windows.

Math:
  For each position l: h_{k+1} = a*h_k + relu(h_k @ W* + u*[l+k]),  h_0 = 0
  where a = 1 - 1/tau, W* = W * (1/tau) (columns), u* = Xp @ W_in* + b*,
  W_in* = W_in * (1/tau), b* = b * (1/tau).  Output = h_8 per position.
  (Uses relu(c*z) = c*relu(z) for c>0 to fold 1/tau into the weights, and
  the fact that the input projection is shared across overlapping windows.)

Sharding: batch dim (8) across the 8 NeuronCores, weights replicated.
On-chip layout is transposed ([d on partitions, positions on free dim]) so
matmuls contract d on the partition axis.
"""

import numpy as np
from contextlib import ExitStack

import concourse.bass as bass
import concourse.tile as tile
from concourse import bacc, mybir
from concourse.bass_utils import run_bass_kernel_spmd

B, L, D, KSIZE = 8, 2048, 256, 8
P = 128
NCORES = 8
CHUNK = 512
NCHUNK = L // CHUNK          # 4
NROWT = L // P               # 16
UCOLS = L + KSIZE - 1        # 2055
DB = D // P                  # 2 d-blocks
F32 = mybir.dt.float32
F32R = mybir.dt.float32r
AF = mybir.ActivationFunctionType
ALU = mybir.AluOpType

_cache = {}


def _build_program():
    nc = bacc.Bacc(
        "TRN2",
        target_bir_lowering=False,
        debug=False,
        enable_asserts=False,
        num_devices=NCORES,
    )
    x_d = nc.dram_tensor("x", (L, D), F32, kind="ExternalInput").ap()
    w_d = nc.dram_tensor("wstar", (D, D), F32R, kind="ExternalInput").ap()
    win_d = nc.dram_tensor("winstar", (D, D), F32R, kind="ExternalInput").ap()
    bs_d = nc.dram_tensor("bstar", (P, DB), F32, kind="ExternalInput").ap()
    a_d = nc.dram_tensor("adecay", (P, DB), F32, kind="ExternalInput").ap()
    id_d = nc.dram_tensor("ident", (P, P), F32, kind="ExternalInput").ap()
    idr_d = nc.dram_tensor("identr", (P, P), F32R, kind="ExternalInput").ap()
    out_d = nc.dram_tensor("out", (L, D), F32, kind="ExternalOutput").ap()

    with tile.TileContext(nc) as tc, ExitStack() as ctx:
        consts = ctx.enter_context(tc.tile_pool(name="consts", bufs=1))
        big = ctx.enter_context(tc.tile_pool(name="big", bufs=1))
        xin = ctx.enter_context(tc.tile_pool(name="xin", bufs=8))
        rp = ctx.enter_context(tc.tile_pool(name="rp", bufs=4))
        outp = ctx.enter_context(tc.tile_pool(name="outp", bufs=4))
        zp = ctx.enter_context(tc.tile_pool(name="zp", bufs=4, space="PSUM"))
        tp = ctx.enter_context(tc.tile_pool(name="tp", bufs=2, space="PSUM"))
        op = ctx.enter_context(tc.tile_pool(name="op", bufs=2, space="PSUM"))

        # --- constants ---
        wt = [consts.tile([P, D], F32R, name=f"wt{i}") for i in range(DB)]
        wint = [consts.tile([P, D], F32R, name=f"wint{i}") for i in range(DB)]
        bst = consts.tile([P, DB], F32, name="bst")
        at = consts.tile([P, DB], F32, name="at")
        ident = consts.tile([P, P], F32, name="ident")
        identr = consts.tile([P, P], F32R, name="identr")
        for i in range(DB):
            nc.sync.dma_start(wt[i][:], w_d[i * P:(i + 1) * P, :])
            nc.sync.dma_start(wint[i][:], win_d[i * P:(i + 1) * P, :])
        nc.sync.dma_start(bst[:], bs_d[:, :])
        nc.sync.dma_start(at[:], a_d[:, :])
        nc.sync.dma_start(ident[:], id_d[:, :])
        nc.sync.dma_start(identr[:], idr_d[:, :])

        # --- persistent buffers ---
        xt = [big.tile([P, L], F32R, name=f"xt{i}") for i in range(DB)]
        ut = [big.tile([P, UCOLS], F32R, name=f"ut{i}") for i in range(DB)]
        hb = [[big.tile([P, L], F32R, name=f"h{s}_{i}") for i in range(DB)]
              for s in range(2)]

        # --- load x and transpose to [d, pos] ---
        for grp in range(NCHUNK):
            xls = []
            for q in range(4):
                c = grp * 4 + q
                xload = xin.tile([P, D], F32, name="xload", tag="xload")
                nc.sync.dma_start(xload[:], x_d[c * P:(c + 1) * P, :])
                xls.append(xload)
            for i in range(DB):
                tpt = tp.tile([P, CHUNK], F32, name="tpt", tag="tpt")
                for q in range(4):
                    nc.tensor.transpose(
                        tpt[:, q * P:(q + 1) * P],
                        xls[q][:, i * P:(i + 1) * P],
                        ident[:],
                    )
                nc.vector.tensor_copy(
                    xt[i][:, grp * CHUNK:(grp + 1) * CHUNK], tpt[:]
                )

        # --- u preamble: u = Xp @ W_in* + b*  (T-layout) ---
        for j in range(DB):
            # pad cols = b* exactly: 0*ident + b* (ident is a safe finite src)
            nc.scalar.activation(
                ut[j][:, 0:KSIZE - 1], ident[:, 0:KSIZE - 1],
                AF.Identity, bias=bst[:, j:j + 1], scale=0.0,
            )
        for grp in range(NCHUNK):
            for j in range(DB):
                zt = zp.tile([P, CHUNK], F32, name="zt", tag="zt")
                for i in range(DB):
                    nc.tensor.matmul(
                        zt[:],
                        lhsT=wint[i][:, j * P:(j + 1) * P],
                        rhs=xt[i][:, grp * CHUNK:(grp + 1) * CHUNK],
                        start=(i == 0),
                        stop=(i == DB - 1),
                    )
                nc.scalar.activation(
                    ut[j][:, KSIZE - 1 + grp * CHUNK:KSIZE - 1 + (grp + 1) * CHUNK],
                    zt[:], AF.Identity, bias=bst[:, j:j + 1], scale=1.0,
                )

        # --- step 0: h1 = relu(u[:, 0:L]) ---
        for j in range(DB):
            for grp in range(NCHUNK):
                nc.scalar.activation(
                    hb[1][j][:, grp * CHUNK:(grp + 1) * CHUNK],
                    ut[j][:, grp * CHUNK:(grp + 1) * CHUNK], AF.Relu,
                )

        # --- steps 1..7 ---
        for k in range(1, KSIZE):
            hc = hb[k % 2]
            hn = hb[(k + 1) % 2]
            for rt in range(NCHUNK):
                cs = rt * CHUNK
                for j in range(DB):
                    zt = zp.tile([P, CHUNK], F32, name="zt", tag="zt")
                    for i in range(DB):
                        nc.tensor.matmul(
                            zt[:],
                            lhsT=wt[i][:, j * P:(j + 1) * P],
                            rhs=hc[i][:, cs:cs + CHUNK],
                            start=(i == 0),
                            stop=False,
                        )
                    nc.tensor.matmul(
                        zt[:],
                        lhsT=identr[:],
                        rhs=ut[j][:, k + cs:k + cs + CHUNK],
                        start=False,
                        stop=True,
                    )
                    r = rp.tile([P, CHUNK], F32, name="r", tag="r")
                    nc.scalar.activation(r[:], zt[:], AF.Relu)
                    nc.vector.scalar_tensor_tensor(
                        out=hn[j][:, cs:cs + CHUNK],
                        in0=hc[j][:, cs:cs + CHUNK],
                        scalar=at[:, j:j + 1],
                        in1=r[:],
                        op0=ALU.mult,
                        op1=ALU.add,
                    )

        # --- output: transpose h8 back to [pos, d] and store ---
        h8 = hb[KSIZE % 2]
        for c in range(NROWT):
            ot = op.tile([P, D], F32R, name="ot", tag="ot")
            for j in range(DB):
                nc.tensor.transpose(
                    ot[:, j * P:(j + 1) * P],
                    h8[j][:, c * P:(c + 1) * P],
                    identr[:],
                )
            st = outp.tile([P, D], F32, name="st", tag="st")
            nc.vector.tensor_copy(st[:], ot[:])
            nc.sync.dma_start(out_d[c * P:(c + 1) * P, :], st[:])

    nc.compile()
    return nc


def get_program():
    if "nc" not in _cache:
        _cache["nc"] = _build_program()
    return _cache["nc"]


def make_in_maps(x, weight, input_weight, bias, tau):
    x = np.asarray(x, dtype=np.float32)
    weight = np.asarray(weight, dtype=np.float32)
    input_weight = np.asarray(input_weight, dtype=np.float32)
    bias = np.asarray(bias, dtype=np.float32).reshape(1, D)
    tau = np.asarray(tau, dtype=np.float32).reshape(1, D)

    inv_tau = 1.0 / tau                       # (1, D)
    a = 1.0 - inv_tau
    wstar = (weight * inv_tau).astype(np.float32)          # scale columns
    winstar = (input_weight * inv_tau).astype(np.float32)
    bstar = (bias * inv_tau).astype(np.float32)
    # per-partition layout (P, DB): col j holds elems [j*P, (j+1)*P)
    bstar_t = np.ascontiguousarray(bstar.reshape(DB, P).T)
    a_t = np.ascontiguousarray(a.reshape(DB, P).T)
    ident = np.eye(P, dtype=np.float32)

    shared = {
        "wstar": np.ascontiguousarray(wstar),
        "winstar": np.ascontiguousarray(winstar),
        "bstar": bstar_t,
        "adecay": a_t,
        "ident": ident,
        "identr": ident,
    }
    return [
        {"x": np.ascontiguousarray(x[b]), **shared} for b in range(NCORES)
    ]


def kernel(x, weight, input_weight, bias, tau, ksize, _trace=False):
    assert int(ksize) == KSIZE
    nc = get_program()
    in_maps = make_in_maps(x, weight, input_weight, bias, tau)
    res = run_bass_kernel_spmd(
        nc, in_maps, core_ids=list(range(NCORES)), trace=_trace
    )
    out = np.stack([res.results[b]["out"] for b in range(NCORES)], axis=0)
    if _trace:
        _cache["last_results"] = res
    return out.astype(np.float32)


# revision 4
# speedup vs baseline: 1.0317x; 1.0317x over previous
"""Trainium2 Bass kernel for nn_LocalRNN: 8-step CTRNN over sliding windows.

Math:
  For each position l: h_{k+1} = a*h_k + relu(h_k @ W* + u*[l+k]),  h_0 = 0
  where a = 1 - 1/tau, W* = W * (1/tau) (columns), u* = Xp @ W_in* + b*,
  W_in* = W_in * (1/tau), b* = b * (1/tau).  Output = h_8 per position.
  (Uses relu(c*z) = c*relu(z) for c>0 to fold 1/tau into the weights, and
  the fact that the input projection is shared across overlapping windows.)

Sharding: batch dim (8) across the 8 NeuronCores, weights replicated.
On-chip layout is transposed ([d on partitions, positions on free dim]) so
matmuls contract d on the partition axis.
"""

import numpy as np
from contextlib import ExitStack

import concourse.bass as bass
import concourse.tile as tile
from concourse import bacc, mybir
from concourse.bass_utils import run_bass_kernel_spmd

B, L, D, KSIZE = 8, 2048, 256, 8
P = 128
NCORES = 8
MMN = 512                    # matmul moving free dim (PSUM bank limit)
WCH = 1024                   # wide chunk for ACT/DVE elementwise ops
NW = L // WCH                # 2
NG = L // MMN                # 4 groups of 512
NROWT = L // P               # 16
UCOLS = L + KSIZE - 1        # 2055
DB = D // P                  # 2 d-blocks
F32 = mybir.dt.float32
F32R = mybir.dt.float32r
AF = mybir.ActivationFunctionType
ALU = mybir.AluOpType

_cache = {}


def _build_program():
    nc = bacc.Bacc(
        "TRN2",
        target_bir_lowering=False,
        debug=False,
        enable_asserts=False,
        num_devices=NCORES,
    )
    x_d = nc.dram_tensor("x", (L, D), F32, kind="ExternalInput").ap()
    w_d = nc.dram_tensor("wstar", (D, D), F32R, kind="ExternalInput").ap()
    win_d = nc.dram_tensor("winstar", (D, D), F32R, kind="ExternalInput").ap()
    bs_d = nc.dram_tensor("bstar", (P, DB), F32, kind="ExternalInput").ap()
    a_d = nc.dram_tensor("adecay", (P, DB), F32, kind="ExternalInput").ap()
    id_d = nc.dram_tensor("ident", (P, P), F32, kind="ExternalInput").ap()
    idr_d = nc.dram_tensor("identr", (P, P), F32R, kind="ExternalInput").ap()
    out_d = nc.dram_tensor("out", (L, D), F32, kind="ExternalOutput").ap()

    with tile.TileContext(nc) as tc, ExitStack() as ctx:
        consts = ctx.enter_context(tc.tile_pool(name="consts", bufs=1))
        big = ctx.enter_context(tc.tile_pool(name="big", bufs=1))
        xin = ctx.enter_context(tc.tile_pool(name="xin", bufs=2))
        rp = ctx.enter_context(tc.tile_pool(name="rp", bufs=3))
        outp = ctx.enter_context(tc.tile_pool(name="outp", bufs=4))
        # single PSUM pool, all tags share slots: [128,1024] slot = 2 banks,
        # bufs=4 -> 8 banks
        zp = ctx.enter_context(tc.tile_pool(name="zp", bufs=4, space="PSUM"))

        # --- constants ---
        wt = [consts.tile([P, D], F32R, name=f"wt{i}") for i in range(DB)]
        wint = [consts.tile([P, D], F32R, name=f"wint{i}") for i in range(DB)]
        bst = consts.tile([P, DB], F32, name="bst")
        at = consts.tile([P, DB], F32, name="at")
        ident = consts.tile([P, P], F32, name="ident")
        identr = consts.tile([P, P], F32R, name="identr")
        for i in range(DB):
            nc.sync.dma_start(wt[i][:], w_d[i * P:(i + 1) * P, :])
            nc.sync.dma_start(wint[i][:], win_d[i * P:(i + 1) * P, :])
        nc.sync.dma_start(bst[:], bs_d[:, :])
        nc.sync.dma_start(at[:], a_d[:, :])
        nc.sync.dma_start(ident[:], id_d[:, :])
        nc.sync.dma_start(identr[:], idr_d[:, :])

        # --- persistent buffers ---
        xt = [big.tile([P, L], F32R, name=f"xt{i}") for i in range(DB)]
        ut = [big.tile([P, UCOLS], F32R, name=f"ut{i}") for i in range(DB)]
        hb = [[big.tile([P, L], F32R, name=f"h{s}_{i}") for i in range(DB)]
              for s in range(2)]

        # u pad cols = b* exactly: 0*ident + b* (ident is a safe finite src)
        for j in range(DB):
            nc.scalar.activation(
                ut[j][:, 0:KSIZE - 1], ident[:, 0:KSIZE - 1],
                AF.Identity, bias=bst[:, j:j + 1], scale=0.0,
            )

        # --- preamble, per 512-wide group: load, transpose, project ---
        for g in range(NG):
            # one big DMA: 512 rows -> [128, 4x256]
            xload = xin.tile([P, 4 * D], F32, name="xload", tag="xload")
            nc.sync.dma_start(
                xload[:].rearrange("p (b d) -> p b d", d=D),
                x_d[g * MMN:(g + 1) * MMN, :].rearrange("(b p) d -> p b d", p=P),
            )
            for i in range(DB):
                tpt = zp.tile([P, MMN], F32, name="tpt", tag="z")
                for q in range(4):
                    nc.tensor.transpose(
                        tpt[:, q * P:(q + 1) * P],
                        xload[:, q * D + i * P:q * D + (i + 1) * P],
                        ident[:],
                    )
                nc.vector.tensor_copy(
                    xt[i][:, g * MMN:(g + 1) * MMN], tpt[:]
                )
            for j in range(DB):
                zt = zp.tile([P, MMN], F32, name="zt", tag="z")
                for i in range(DB):
                    nc.tensor.matmul(
                        zt[:],
                        lhsT=wint[i][:, j * P:(j + 1) * P],
                        rhs=xt[i][:, g * MMN:(g + 1) * MMN],
                        start=(i == 0),
                        stop=(i == DB - 1),
                    )
                nc.scalar.activation(
                    ut[j][:, KSIZE - 1 + g * MMN:KSIZE - 1 + (g + 1) * MMN],
                    zt[:], AF.Identity, bias=bst[:, j:j + 1], scale=1.0,
                )

        # --- step 0: h1 = relu(u[:, 0:L]) ---
        for j in range(DB):
            for c in range(NW):
                nc.scalar.activation(
                    hb[1][j][:, c * WCH:(c + 1) * WCH],
                    ut[j][:, c * WCH:(c + 1) * WCH], AF.Relu,
                )

        # --- steps 1..7 (wide 1024-col chunks; matmuls in 512 halves) ---
        for k in range(1, KSIZE):
            hc = hb[k % 2]
            hn = hb[(k + 1) % 2]
            for c in range(NW):
                cs = c * WCH
                for j in range(DB):
                    zt = zp.tile([P, WCH], F32, name="zt", tag="z")
                    for h in range(2):
                        hs = cs + h * MMN
                        zh = zt[:, h * MMN:(h + 1) * MMN]
                        for i in range(DB):
                            nc.tensor.matmul(
                                zh,
                                lhsT=wt[i][:, j * P:(j + 1) * P],
                                rhs=hc[i][:, hs:hs + MMN],
                                start=(i == 0),
                                stop=False,
                            )
                        nc.tensor.matmul(
                            zh,
                            lhsT=identr[:],
                            rhs=ut[j][:, k + hs:k + hs + MMN],
                            start=False,
                            stop=True,
                        )
                    r = rp.tile([P, WCH], F32, name="r", tag="r")
                    nc.scalar.activation(r[:], zt[:], AF.Relu)
                    nc.vector.scalar_tensor_tensor(
                        out=hn[j][:, cs:cs + WCH],
                        in0=hc[j][:, cs:cs + WCH],
                        scalar=at[:, j:j + 1],
                        in1=r[:],
                        op0=ALU.mult,
                        op1=ALU.add,
                    )

        # --- output: transpose h8 back to [pos, d]; 2 row-chunks per tile ---
        h8 = hb[KSIZE % 2]
        for c2 in range(NROWT // 2):
            ot = zp.tile([P, MMN], F32R, name="ot", tag="z")
            for q in range(2):
                c = 2 * c2 + q
                for j in range(DB):
                    nc.tensor.transpose(
                        ot[:, q * D + j * P:q * D + (j + 1) * P],
                        h8[j][:, c * P:(c + 1) * P],
                        identr[:],
                    )
            st = outp.tile([P, MMN], F32, name="st", tag="st")
            nc.scalar.copy(st[:], ot[:])
            nc.sync.dma_start(
                out_d[c2 * 2 * P:(c2 + 1) * 2 * P, :].rearrange(
                    "(b p) d -> p b d", p=P),
                st[:].rearrange("p (b d) -> p b d", d=D),
            )

    nc.compile()
    return nc


def get_program():
    if "nc" not in _cache:
        _cache["nc"] = _build_program()
    return _cache["nc"]


def make_in_maps(x, weight, input_weight, bias, tau):
    x = np.asarray(x, dtype=np.float32)
    weight = np.asarray(weight, dtype=np.float32)
    input_weight = np.asarray(input_weight, dtype=np.float32)
    bias = np.asarray(bias, dtype=np.float32).reshape(1, D)
    tau = np.asarray(tau, dtype=np.float32).reshape(1, D)

    inv_tau = 1.0 / tau                       # (1, D)
    a = 1.0 - inv_tau
    wstar = (weight * inv_tau).astype(np.float32)          # scale columns
    winstar = (input_weight * inv_tau).astype(np.float32)
    bstar = (bias * inv_tau).astype(np.float32)
    # per-partition layout (P, DB): col j holds elems [j*P, (j+1)*P)
    bstar_t = np.ascontiguousarray(bstar.reshape(DB, P).T)
    a_t = np.ascontiguousarray(a.reshape(DB, P).T)
    ident = np.eye(P, dtype=np.float32)

    shared = {
        "wstar": np.ascontiguousarray(wstar),
        "winstar": np.ascontiguousarray(winstar),
        "bstar": bstar_t,
        "adecay": a_t,
        "ident": ident,
        "identr": ident,
    }
    return [
        {"x": np.ascontiguousarray(x[b]), **shared} for b in range(NCORES)
    ]


def kernel(x, weight, input_weight, bias, tau, ksize, _trace=False):
    assert int(ksize) == KSIZE
    nc = get_program()
    in_maps = make_in_maps(x, weight, input_weight, bias, tau)
    res = run_bass_kernel_spmd(
        nc, in_maps, core_ids=list(range(NCORES)), trace=_trace
    )
    out = np.stack([res.results[b]["out"] for b in range(NCORES)], axis=0)
    if _trace:
        _cache["last_results"] = res
    return out.astype(np.float32)


# revision 5
# speedup vs baseline: 1.1404x; 1.1054x over previous
"""Trainium2 Bass kernel for nn_LocalRNN: 8-step CTRNN over sliding windows.

Math:
  For each position l: h_{k+1} = a*h_k + relu(h_k @ W* + u*[l+k]),  h_0 = 0
  where a = 1 - 1/tau, W* = W * (1/tau) (columns), u* = Xp @ W_in* + b*,
  W_in* = W_in * (1/tau), b* = b * (1/tau).  Output = h_8 per position.
  (Uses relu(c*z) = c*relu(z) for c>0 to fold 1/tau into the weights, and
  the fact that the input projection is shared across overlapping windows.)

Sharding: batch dim (8) across the 8 NeuronCores, weights replicated.
On-chip layout is transposed ([d on partitions, positions on free dim]) so
matmuls contract d on the partition axis.
"""

import numpy as np
from contextlib import ExitStack

import concourse.bass as bass
import concourse.tile as tile
from concourse import bacc, mybir
from concourse.bass_utils import run_bass_kernel_spmd

B, L, D, KSIZE = 8, 2048, 256, 8
P = 128
NCORES = 8
MMN = 512                    # matmul moving free dim (PSUM bank limit)
WCH = 1024                   # wide chunk for ACT/DVE elementwise ops
NW = L // WCH                # 2
NG = L // MMN                # 4 groups of 512
NROWT = L // P               # 16
UCOLS = L + KSIZE - 1        # 2055
PAD = KSIZE - 1              # 7
DB = D // P                  # 2 d-blocks
F32 = mybir.dt.float32
F32R = mybir.dt.float32r
AF = mybir.ActivationFunctionType
ALU = mybir.AluOpType

# packed f32r consts blob: wt0|wt1|wint0|wint1|identr  (columns)
CR_WT = [0, D]
CR_WIN = [2 * D, 3 * D]
CR_ID = 4 * D
CR_COLS = 4 * D + P
# packed f32 consts blob: bst|at|ident
CF_BST = 0
CF_AT = DB
CF_ID = 2 * DB
CF_COLS = 2 * DB + P

_cache = {}


def _build_program():
    nc = bacc.Bacc(
        "TRN2",
        target_bir_lowering=False,
        debug=False,
        enable_asserts=False,
        num_devices=NCORES,
    )
    x_d = nc.dram_tensor("x", (L, D), F32R, kind="ExternalInput").ap()
    cr_d = nc.dram_tensor("constsr", (P, CR_COLS), F32R, kind="ExternalInput").ap()
    cf_d = nc.dram_tensor("constsf", (P, CF_COLS), F32, kind="ExternalInput").ap()
    out_d = nc.dram_tensor("out", (L, D), F32, kind="ExternalOutput").ap()

    with tile.TileContext(nc) as tc, ExitStack() as ctx:
        consts = ctx.enter_context(tc.tile_pool(name="consts", bufs=1))
        big = ctx.enter_context(tc.tile_pool(name="big", bufs=1))
        xin = ctx.enter_context(tc.tile_pool(name="xin", bufs=2))
        rp = ctx.enter_context(tc.tile_pool(name="rp", bufs=3))
        outp = ctx.enter_context(tc.tile_pool(name="outp", bufs=4))
        # single PSUM pool, all tags share slots: [128,1024] slot = 2 banks,
        # bufs=4 -> 8 banks
        zp = ctx.enter_context(tc.tile_pool(name="zp", bufs=4, space="PSUM"))

        # --- constants: two packed DMAs on gpsimd (parallel with sync) ---
        cr = consts.tile([P, CR_COLS], F32R, name="cr")
        cf = consts.tile([P, CF_COLS], F32, name="cf")
        nc.gpsimd.dma_start(cr[:], cr_d[:, :])
        nc.gpsimd.dma_start(cf[:], cf_d[:, :])
        wt = [cr[:, i * D:(i + 1) * D] for i in range(DB)]
        wint = [cr[:, (2 + i) * D:(3 + i) * D] for i in range(DB)]
        identr = cr[:, CR_ID:CR_ID + P]
        bst = cf[:, CF_BST:CF_BST + DB]
        at = cf[:, CF_AT:CF_AT + DB]

        # --- persistent buffers ---
        xt = [big.tile([P, L], F32R, name=f"xt{i}") for i in range(DB)]
        ut = [big.tile([P, UCOLS], F32R, name=f"ut{i}") for i in range(DB)]
        hb = [[big.tile([P, L], F32R, name=f"h{s}_{i}") for i in range(DB)]
              for s in range(2)]
        h1 = hb[1]

        # u pad cols + h1 pad cols (also warms the ACT table early):
        # u[:, :7] = b*, h1[:, :7] = relu(b*)
        for j in range(DB):
            nc.scalar.activation(
                ut[j][:, 0:PAD], cf[:, 0:PAD],
                AF.Identity, bias=bst[:, j:j + 1], scale=0.0,
            )
            nc.scalar.activation(
                h1[j][:, 0:PAD], cf[:, 0:PAD],
                AF.Relu, bias=bst[:, j:j + 1], scale=0.0,
            )

        # --- input: 4 big DMAs on sync ---
        xls = []
        for g in range(NG):
            xload = xin.tile([P, 4 * D], F32R, name="xload", tag="xload",
                             bufs=4)
            nc.sync.dma_start(
                xload[:].rearrange("p (b d) -> p b d", d=D),
                x_d[g * MMN:(g + 1) * MMN, :].rearrange("(b p) d -> p b d", p=P),
            )
            xls.append(xload)

        # --- transpose x to [d, pos] ---
        for g in range(NG):
            for i in range(DB):
                tpt = zp.tile([P, MMN], F32R, name="tpt", tag="z")
                for q in range(4):
                    nc.tensor.transpose(
                        tpt[:, q * P:(q + 1) * P],
                        xls[g][:, q * D + i * P:q * D + (i + 1) * P],
                        identr,
                    )
                nc.vector.tensor_copy(
                    xt[i][:, g * MMN:(g + 1) * MMN], tpt[:]
                )

        # --- u projection, wide tiles; h1 (ACT) and u (DVE) read PSUM ---
        for gw in range(2):
            for j in range(DB):
                zt = zp.tile([P, WCH], F32, name="zu", tag="z")
                for half in range(2):
                    g = 2 * gw + half
                    zh = zt[:, half * MMN:(half + 1) * MMN]
                    for i in range(DB):
                        nc.tensor.matmul(
                            zh,
                            lhsT=wint[i][:, j * P:(j + 1) * P],
                            rhs=xt[i][:, g * MMN:(g + 1) * MMN],
                            start=(i == 0),
                            stop=(i == DB - 1),
                        )
                # h1 positions [7+1024gw, min(7+1024(gw+1), 2048))
                hw = WCH if gw == 0 else WCH - PAD
                nc.scalar.activation(
                    h1[j][:, PAD + gw * WCH:PAD + gw * WCH + hw],
                    zt[:, 0:hw], AF.Relu, bias=bst[:, j:j + 1], scale=1.0,
                )
                nc.vector.tensor_scalar(
                    out=ut[j][:, PAD + gw * WCH:PAD + (gw + 1) * WCH],
                    in0=zt[:],
                    scalar1=bst[:, j:j + 1],
                    scalar2=None,
                    op0=ALU.add,
                )

        # --- steps 1..7 (wide 1024-col chunks; matmuls in 512 halves) ---
        for k in range(1, KSIZE):
            hc = hb[k % 2]
            hn = hb[(k + 1) % 2]
            for c in range(NW):
                cs = c * WCH
                for j in range(DB):
                    zt = zp.tile([P, WCH], F32, name="zt", tag="z")
                    for h in range(2):
                        hs = cs + h * MMN
                        zh = zt[:, h * MMN:(h + 1) * MMN]
                        for i in range(DB):
                            nc.tensor.matmul(
                                zh,
                                lhsT=wt[i][:, j * P:(j + 1) * P],
                                rhs=hc[i][:, hs:hs + MMN],
                                start=(i == 0),
                                stop=False,
                            )
                        nc.tensor.matmul(
                            zh,
                            lhsT=identr,
                            rhs=ut[j][:, k + hs:k + hs + MMN],
                            start=False,
                            stop=True,
                        )
                    r = rp.tile([P, WCH], F32, name="r", tag="r")
                    nc.scalar.activation(r[:], zt[:], AF.Relu)
                    nc.vector.scalar_tensor_tensor(
                        out=hn[j][:, cs:cs + WCH],
                        in0=hc[j][:, cs:cs + WCH],
                        scalar=at[:, j:j + 1],
                        in1=r[:],
                        op0=ALU.mult,
                        op1=ALU.add,
                    )

        # --- output: transpose h8 back to [pos, d]; 2 row-chunks per tile ---
        h8 = hb[KSIZE % 2]
        for c2 in range(NROWT // 2):
            ot = zp.tile([P, MMN], F32R, name="ot", tag="z")
            for q in range(2):
                c = 2 * c2 + q
                for j in range(DB):
                    nc.tensor.transpose(
                        ot[:, q * D + j * P:q * D + (j + 1) * P],
                        h8[j][:, c * P:(c + 1) * P],
                        identr,
                    )
            st = outp.tile([P, MMN], F32, name="st", tag="st")
            nc.scalar.copy(st[:], ot[:])
            nc.sync.dma_start(
                out_d[c2 * 2 * P:(c2 + 1) * 2 * P, :].rearrange(
                    "(b p) d -> p b d", p=P),
                st[:].rearrange("p (b d) -> p b d", d=D),
            )

    nc.compile()
    return nc


def get_program():
    if "nc" not in _cache:
        _cache["nc"] = _build_program()
    return _cache["nc"]


def make_in_maps(x, weight, input_weight, bias, tau):
    x = np.asarray(x, dtype=np.float32)
    weight = np.asarray(weight, dtype=np.float32)
    input_weight = np.asarray(input_weight, dtype=np.float32)
    bias = np.asarray(bias, dtype=np.float32).reshape(1, D)
    tau = np.asarray(tau, dtype=np.float32).reshape(1, D)

    inv_tau = 1.0 / tau                       # (1, D)
    a = 1.0 - inv_tau
    wstar = (weight * inv_tau).astype(np.float32)          # scale columns
    winstar = (input_weight * inv_tau).astype(np.float32)
    bstar = (bias * inv_tau).astype(np.float32)
    # per-partition layout (P, DB): col j holds elems [j*P, (j+1)*P)
    bstar_t = bstar.reshape(DB, P).T
    a_t = a.reshape(DB, P).T
    ident = np.eye(P, dtype=np.float32)

    cr = np.concatenate(
        [wstar[0:P, :], wstar[P:D, :],
         winstar[0:P, :], winstar[P:D, :], ident], axis=1)
    cf = np.concatenate([bstar_t, a_t, ident], axis=1)

    shared = {
        "constsr": np.ascontiguousarray(cr),
        "constsf": np.ascontiguousarray(cf),
    }
    return [
        {"x": np.ascontiguousarray(x[b]), **shared} for b in range(NCORES)
    ]


def kernel(x, weight, input_weight, bias, tau, ksize, _trace=False):
    assert int(ksize) == KSIZE
    nc = get_program()
    in_maps = make_in_maps(x, weight, input_weight, bias, tau)
    res = run_bass_kernel_spmd(
        nc, in_maps, core_ids=list(range(NCORES)), trace=_trace
    )
    out = np.stack([res.results[b]["out"] for b in range(NCORES)], axis=0)
    if _trace:
        _cache["last_results"] = res
    return out.astype(np.float32)


# revision 8
# speedup vs baseline: 1.1639x; 1.0206x over previous
"""Trainium2 Bass kernel for nn_LocalRNN: 8-step CTRNN over sliding windows.

Math:
  For each position l: h_{k+1} = a*h_k + relu(h_k @ W* + u*[l+k]),  h_0 = 0
  where a = 1 - 1/tau, W* = W * (1/tau) (columns), u* = Xp @ W_in* + b*,
  W_in* = W_in * (1/tau), b* = b * (1/tau).  Output = h_8 per position.
  (Uses relu(c*z) = c*relu(z) for c>0 to fold 1/tau into the weights, and
  the fact that the input projection is shared across overlapping windows.)

Sharding: batch dim (8) across the 8 NeuronCores, weights replicated.
On-chip layout is transposed ([d on partitions, positions on free dim]) so
matmuls contract d on the partition axis; the host uploads x pre-transposed
and transposes the [d, pos] output back (layout marshalling only).
"""

import numpy as np
from contextlib import ExitStack

import concourse.bass as bass
import concourse.tile as tile
from concourse import bacc, mybir
from concourse.bass_utils import run_bass_kernel_spmd

B, L, D, KSIZE = 8, 2048, 256, 8
P = 128
NCORES = 8
MMN = 512                    # matmul moving free dim (PSUM bank limit)
WCH = 1024                   # wide chunk for ACT/DVE elementwise ops
NW = L // WCH                # 2
NG = L // MMN                # 4 groups of 512
UCOLS = L + KSIZE - 1        # 2055
PAD = KSIZE - 1              # 7
DB = D // P                  # 2 d-blocks
F32 = mybir.dt.float32
F32R = mybir.dt.float32r
AF = mybir.ActivationFunctionType
ALU = mybir.AluOpType

# packed f32r consts blob: wt0|wt1|wint0|wint1|identr  (columns)
CR_COLS = 4 * D + P
CR_ID = 4 * D
# packed f32 consts blob: bst|at|pad src
CF_COLS = 2 * DB + P
_cache = {}


def _build_program():
    nc = bacc.Bacc(
        "TRN2",
        target_bir_lowering=False,
        debug=False,
        enable_asserts=False,
        num_devices=NCORES,
    )
    # x uploaded pre-transposed: (D, L), row d -> [d, positions]
    x_d = nc.dram_tensor("xt", (D, L), F32R, kind="ExternalInput").ap()
    cr_d = nc.dram_tensor("constsr", (P, CR_COLS), F32R, kind="ExternalInput").ap()
    cf_d = nc.dram_tensor("constsf", (P, CF_COLS), F32, kind="ExternalInput").ap()
    # output in T-layout: (D, L); host transposes back
    out_d = nc.dram_tensor("out", (D, L), F32R, kind="ExternalOutput").ap()

    with tile.TileContext(nc) as tc, ExitStack() as ctx:
        consts = ctx.enter_context(tc.tile_pool(name="consts", bufs=1))
        big = ctx.enter_context(tc.tile_pool(name="big", bufs=1))
        rp = ctx.enter_context(tc.tile_pool(name="rp", bufs=3))
        # single PSUM pool, all tags share slots: [128,1024] slot = 2 banks,
        # bufs=4 -> 8 banks
        zp = ctx.enter_context(tc.tile_pool(name="zp", bufs=4, space="PSUM"))

        # --- PE warmup: dummy matmuls on garbage data to engage HAM early ---
        dummy_f = big.tile([P, MMN], F32, name="dummy_f")
        dummy = big.tile([P, MMN], F32R, name="dummy")
        nc.vector.memset(dummy_f[:], 0.0)
        nc.vector.tensor_copy(dummy[:], dummy_f[:])
        warm = zp.tile([P, MMN], F32, name="warm", tag="z")
        for _ in range(16):
            nc.tensor.matmul(warm[:], lhsT=dummy[:, 0:P], rhs=dummy[:],
                             start=True, stop=True)

        # --- constants: two packed DMAs on gpsimd (parallel with sync) ---
        cr = consts.tile([P, CR_COLS], F32R, name="cr")
        cf = consts.tile([P, CF_COLS], F32, name="cf")
        nc.gpsimd.dma_start(cr[:], cr_d[:, :])
        nc.gpsimd.dma_start(cf[:], cf_d[:, :])
        wt = [cr[:, i * D:(i + 1) * D] for i in range(DB)]
        wint = [cr[:, (2 + i) * D:(3 + i) * D] for i in range(DB)]
        identr = cr[:, CR_ID:CR_ID + P]
        bst = cf[:, 0:DB]
        at = cf[:, DB:2 * DB]

        # --- persistent buffers: d-blocks side by side in one tile ---
        xt_all = big.tile([P, DB * L], F32R, name="xt_all")
        xt = [xt_all[:, i * L:(i + 1) * L] for i in range(DB)]
        ut = [big.tile([P, UCOLS], F32R, name=f"ut{i}") for i in range(DB)]
        hball = [big.tile([P, DB * L], F32R, name=f"hb{s}") for s in range(2)]
        hb = [[hball[s][:, i * L:(i + 1) * L] for i in range(DB)]
              for s in range(2)]
        h1 = hb[1]

        # u pad cols + h1 pad cols (also warms the ACT table early):
        # u[:, :7] = b*, h1[:, :7] = relu(b*)
        for j in range(DB):
            nc.scalar.activation(
                ut[j][:, 0:PAD], cf[:, 0:PAD],
                AF.Identity, bias=bst[:, j:j + 1], scale=0.0,
            )
            nc.scalar.activation(
                h1[j][:, 0:PAD], cf[:, 0:PAD],
                AF.Relu, bias=bst[:, j:j + 1], scale=0.0,
            )

        # --- input: 4 DMAs, alternating HWDGE engines, both d-blocks each ---
        for g in range(NG):
            eng = nc.sync if g % 2 == 0 else nc.scalar
            eng.dma_start(
                xt_all[:].rearrange("p (i c) -> p i c", i=DB)[
                    :, :, g * MMN:(g + 1) * MMN],
                x_d.rearrange("(i p) c -> p i c", p=P)[
                    :, :, g * MMN:(g + 1) * MMN],
            )

        # --- u projection, wide tiles; h1 (ACT) and u (DVE) read PSUM ---
        for gw in range(2):
            for j in range(DB):
                zt = zp.tile([P, WCH], F32, name="zu", tag="z")
                for half in range(2):
                    g = 2 * gw + half
                    zh = zt[:, half * MMN:(half + 1) * MMN]
                    for i in range(DB):
                        nc.tensor.matmul(
                            zh,
                            lhsT=wint[i][:, j * P:(j + 1) * P],
                            rhs=xt[i][:, g * MMN:(g + 1) * MMN],
                            start=(i == 0),
                            stop=(i == DB - 1),
                        )
                # h1 positions [7+1024gw, min(7+1024(gw+1), 2048))
                hw = WCH if gw == 0 else WCH - PAD
                nc.scalar.activation(
                    h1[j][:, PAD + gw * WCH:PAD + gw * WCH + hw],
                    zt[:, 0:hw], AF.Relu, bias=bst[:, j:j + 1], scale=1.0,
                )
                nc.vector.tensor_scalar(
                    out=ut[j][:, PAD + gw * WCH:PAD + (gw + 1) * WCH],
                    in0=zt[:],
                    scalar1=bst[:, j:j + 1],
                    scalar2=None,
                    op0=ALU.add,
                )

        # --- steps 1..7 (wide 1024-col chunks; matmuls in 512 halves) ---
        for k in range(1, KSIZE):
            hc = hb[k % 2]
            hn = hb[(k + 1) % 2]
            for c in range(NW):
                cs = c * WCH
                for j in range(DB):
                    zt = zp.tile([P, WCH], F32, name="zt", tag="z")
                    for h in range(2):
                        hs = cs + h * MMN
                        zh = zt[:, h * MMN:(h + 1) * MMN]
                        for i in range(DB):
                            nc.tensor.matmul(
                                zh,
                                lhsT=wt[i][:, j * P:(j + 1) * P],
                                rhs=hc[i][:, hs:hs + MMN],
                                start=(i == 0),
                                stop=False,
                            )
                        nc.tensor.matmul(
                            zh,
                            lhsT=identr,
                            rhs=ut[j][:, k + hs:k + hs + MMN],
                            start=False,
                            stop=True,
                        )
                    r = rp.tile([P, WCH], F32, name="r", tag="r")
                    nc.scalar.activation(r[:], zt[:], AF.Relu)
                    nc.vector.scalar_tensor_tensor(
                        out=hn[j][:, cs:cs + WCH],
                        in0=hc[j][:, cs:cs + WCH],
                        scalar=at[:, j:j + 1],
                        in1=r[:],
                        op0=ALU.mult,
                        op1=ALU.add,
                    )

        # --- output: h8 is [d, pos]; single DMA, host transposes back ---
        h8all = hball[KSIZE % 2]
        nc.sync.dma_start(
            out_d.rearrange("(i p) c -> p i c", p=P),
            h8all[:].rearrange("p (i c) -> p i c", i=DB),
        )

    nc.compile()
    return nc


def get_program():
    if "nc" not in _cache:
        _cache["nc"] = _build_program()
    return _cache["nc"]


def make_in_maps(x, weight, input_weight, bias, tau):
    x = np.asarray(x, dtype=np.float32)
    weight = np.asarray(weight, dtype=np.float32)
    input_weight = np.asarray(input_weight, dtype=np.float32)
    bias = np.asarray(bias, dtype=np.float32).reshape(1, D)
    tau = np.asarray(tau, dtype=np.float32).reshape(1, D)

    inv_tau = 1.0 / tau                       # (1, D)
    a = 1.0 - inv_tau
    wstar = (weight * inv_tau).astype(np.float32)          # scale columns
    winstar = (input_weight * inv_tau).astype(np.float32)
    bstar = (bias * inv_tau).astype(np.float32)
    # per-partition layout (P, DB): col j holds elems [j*P, (j+1)*P)
    bstar_t = bstar.reshape(DB, P).T
    a_t = a.reshape(DB, P).T
    ident = np.eye(P, dtype=np.float32)

    cr = np.concatenate(
        [wstar[0:P, :], wstar[P:D, :],
         winstar[0:P, :], winstar[P:D, :], ident], axis=1)
    cf = np.concatenate([bstar_t, a_t, np.zeros((P, P), np.float32)], axis=1)

    shared = {
        "constsr": np.ascontiguousarray(cr),
        "constsf": np.ascontiguousarray(cf),
    }
    return [
        {"xt": np.ascontiguousarray(x[b].T), **shared} for b in range(NCORES)
    ]


def kernel(x, weight, input_weight, bias, tau, ksize, _trace=False):
    assert int(ksize) == KSIZE
    nc = get_program()
    in_maps = make_in_maps(x, weight, input_weight, bias, tau)
    res = run_bass_kernel_spmd(
        nc, in_maps, core_ids=list(range(NCORES)), trace=_trace
    )
    out = np.stack(
        [np.ascontiguousarray(res.results[b]["out"].T) for b in range(NCORES)],
        axis=0,
    )
    if _trace:
        _cache["last_results"] = res
    return out.astype(np.float32)


# revision 9
# speedup vs baseline: 1.1888x; 1.0214x over previous
"""Trainium2 Bass kernel for nn_LocalRNN: 8-step CTRNN over sliding windows.

Math:
  For each position l: h_{k+1} = a*h_k + relu(h_k @ W* + u*[l+k]),  h_0 = 0
  where a = 1 - 1/tau, W* = W * (1/tau) (columns), u* = Xp @ W_in* + b*,
  W_in* = W_in * (1/tau), b* = b * (1/tau).  Output = h_8 per position.
  (Uses relu(c*z) = c*relu(z) for c>0 to fold 1/tau into the weights, and
  the fact that the input projection is shared across overlapping windows.)

Sharding: batch dim (8) across the 8 NeuronCores, weights replicated.
On-chip layout is transposed ([d on partitions, positions on free dim]) so
matmuls contract d on the partition axis; the host uploads x pre-transposed
and transposes the [d, pos] output back (layout marshalling only).
"""

import numpy as np
from contextlib import ExitStack

import concourse.bass as bass
import concourse.tile as tile
from concourse import bacc, mybir
from concourse.bass_utils import run_bass_kernel_spmd

B, L, D, KSIZE = 8, 2048, 256, 8
P = 128
NCORES = 8
MMN = 512                    # matmul moving free dim (PSUM bank limit)
WCH = 1024                   # wide chunk for ACT/DVE elementwise ops
NW = L // WCH                # 2
NG = L // MMN                # 4 groups of 512
UCOLS = L + KSIZE - 1        # 2055
PAD = KSIZE - 1              # 7
DB = D // P                  # 2 d-blocks
F32 = mybir.dt.float32
F32R = mybir.dt.float32r
AF = mybir.ActivationFunctionType
ALU = mybir.AluOpType

# packed f32r consts blob: wt0|wt1|wint0|wint1|identr  (columns)
CR_COLS = 4 * D + P
CR_ID = 4 * D
# packed f32 consts blob: bst|at|pad src
CF_COLS = 2 * DB + P
_cache = {}


def _build_program():
    nc = bacc.Bacc(
        "TRN2",
        target_bir_lowering=False,
        debug=False,
        enable_asserts=False,
        num_devices=NCORES,
    )
    # x uploaded pre-transposed: (D, L), row d -> [d, positions]
    x_d = nc.dram_tensor("xt", (D, L), F32R, kind="ExternalInput").ap()
    cr_d = nc.dram_tensor("constsr", (P, CR_COLS), F32R, kind="ExternalInput").ap()
    cf_d = nc.dram_tensor("constsf", (P, CF_COLS), F32, kind="ExternalInput").ap()
    # output in T-layout: (D, L); host transposes back
    out_d = nc.dram_tensor("out", (D, L), F32R, kind="ExternalOutput").ap()

    with tile.TileContext(nc) as tc, ExitStack() as ctx:
        consts = ctx.enter_context(tc.tile_pool(name="consts", bufs=1))
        big = ctx.enter_context(tc.tile_pool(name="big", bufs=1))
        rp = ctx.enter_context(tc.tile_pool(name="rp", bufs=3))
        # single PSUM pool, all tags share slots: [128,1024] slot = 2 banks,
        # bufs=4 -> 8 banks
        zp = ctx.enter_context(tc.tile_pool(name="zp", bufs=4, space="PSUM"))

        # --- PE warmup: dummy matmuls on garbage data to engage HAM early ---
        dummy_f = big.tile([P, MMN], F32, name="dummy_f")
        dummy = big.tile([P, MMN], F32R, name="dummy")
        nc.vector.memset(dummy_f[:], 0.0)
        nc.vector.tensor_copy(dummy[:], dummy_f[:])
        warm = zp.tile([P, MMN], F32, name="warm", tag="z")
        for _ in range(6):
            nc.tensor.matmul(warm[:], lhsT=dummy[:, 0:P], rhs=dummy[:],
                             start=True, stop=True)

        # --- constants: two packed DMAs on gpsimd (parallel with sync) ---
        cr = consts.tile([P, CR_COLS], F32R, name="cr")
        cf = consts.tile([P, CF_COLS], F32, name="cf")
        nc.sync.dma_start(cr[:], cr_d[:, :])
        nc.scalar.dma_start(cf[:], cf_d[:, :])
        wt = [cr[:, i * D:(i + 1) * D] for i in range(DB)]
        wint = [cr[:, (2 + i) * D:(3 + i) * D] for i in range(DB)]
        identr = cr[:, CR_ID:CR_ID + P]
        bst = cf[:, 0:DB]
        at = cf[:, DB:2 * DB]

        # --- persistent buffers: d-blocks side by side in one tile ---
        xt_all = big.tile([P, DB * L], F32R, name="xt_all")
        xt = [xt_all[:, i * L:(i + 1) * L] for i in range(DB)]
        ut = [big.tile([P, UCOLS], F32R, name=f"ut{i}") for i in range(DB)]
        hball = [big.tile([P, DB * L], F32R, name=f"hb{s}") for s in range(2)]
        hb = [[hball[s][:, i * L:(i + 1) * L] for i in range(DB)]
              for s in range(2)]
        h1 = hb[1]

        # u pad cols + h1 pad cols (also warms the ACT table early):
        # u[:, :7] = b*, h1[:, :7] = relu(b*)
        for j in range(DB):
            nc.scalar.activation(
                ut[j][:, 0:PAD], cf[:, 0:PAD],
                AF.Identity, bias=bst[:, j:j + 1], scale=0.0,
            )
            nc.scalar.activation(
                h1[j][:, 0:PAD], cf[:, 0:PAD],
                AF.Relu, bias=bst[:, j:j + 1], scale=0.0,
            )

        # --- input: 4 DMAs, alternating HWDGE engines, both d-blocks each ---
        for g in range(NG):
            eng = nc.sync if g % 2 == 0 else nc.scalar
            eng.dma_start(
                xt_all[:].rearrange("p (i c) -> p i c", i=DB)[
                    :, :, g * MMN:(g + 1) * MMN],
                x_d.rearrange("(i p) c -> p i c", p=P)[
                    :, :, g * MMN:(g + 1) * MMN],
            )

        # --- u projection, wide tiles; h1 (ACT) and u (DVE) read PSUM ---
        for gw in range(2):
            for j in range(DB):
                zt = zp.tile([P, WCH], F32, name="zu", tag="z")
                for half in range(2):
                    g = 2 * gw + half
                    zh = zt[:, half * MMN:(half + 1) * MMN]
                    for i in range(DB):
                        nc.tensor.matmul(
                            zh,
                            lhsT=wint[i][:, j * P:(j + 1) * P],
                            rhs=xt[i][:, g * MMN:(g + 1) * MMN],
                            start=(i == 0),
                            stop=(i == DB - 1),
                        )
                # h1 positions [7+1024gw, min(7+1024(gw+1), 2048))
                hw = WCH if gw == 0 else WCH - PAD
                nc.scalar.activation(
                    h1[j][:, PAD + gw * WCH:PAD + gw * WCH + hw],
                    zt[:, 0:hw], AF.Relu, bias=bst[:, j:j + 1], scale=1.0,
                )
                nc.vector.tensor_scalar(
                    out=ut[j][:, PAD + gw * WCH:PAD + (gw + 1) * WCH],
                    in0=zt[:],
                    scalar1=bst[:, j:j + 1],
                    scalar2=None,
                    op0=ALU.add,
                )

        # --- steps 1..7 (wide 1024-col chunks; matmuls in 512 halves) ---
        for k in range(1, KSIZE):
            hc = hb[k % 2]
            hn = hb[(k + 1) % 2]
            for c in range(NW):
                cs = c * WCH
                for j in range(DB):
                    zt = zp.tile([P, WCH], F32, name="zt", tag="z")
                    for h in range(2):
                        hs = cs + h * MMN
                        zh = zt[:, h * MMN:(h + 1) * MMN]
                        for i in range(DB):
                            nc.tensor.matmul(
                                zh,
                                lhsT=wt[i][:, j * P:(j + 1) * P],
                                rhs=hc[i][:, hs:hs + MMN],
                                start=(i == 0),
                                stop=False,
                            )
                        nc.tensor.matmul(
                            zh,
                            lhsT=identr,
                            rhs=ut[j][:, k + hs:k + hs + MMN],
                            start=False,
                            stop=True,
                        )
                    r = rp.tile([P, WCH], F32, name="r", tag="r")
                    nc.scalar.activation(r[:], zt[:], AF.Relu)
                    nc.vector.scalar_tensor_tensor(
                        out=hn[j][:, cs:cs + WCH],
                        in0=hc[j][:, cs:cs + WCH],
                        scalar=at[:, j:j + 1],
                        in1=r[:],
                        op0=ALU.mult,
                        op1=ALU.add,
                    )

        # --- output: h8 is [d, pos]; single DMA, host transposes back ---
        h8all = hball[KSIZE % 2]
        for g in range(NG):
            eng = nc.sync if g % 2 == 0 else nc.scalar
            eng.dma_start(
                out_d.rearrange("(i p) c -> p i c", p=P)[
                    :, :, g * MMN:(g + 1) * MMN],
                h8all[:].rearrange("p (i c) -> p i c", i=DB)[
                    :, :, g * MMN:(g + 1) * MMN],
            )

    nc.compile()
    return nc


def get_program():
    if "nc" not in _cache:
        _cache["nc"] = _build_program()
    return _cache["nc"]


def make_in_maps(x, weight, input_weight, bias, tau):
    x = np.asarray(x, dtype=np.float32)
    weight = np.asarray(weight, dtype=np.float32)
    input_weight = np.asarray(input_weight, dtype=np.float32)
    bias = np.asarray(bias, dtype=np.float32).reshape(1, D)
    tau = np.asarray(tau, dtype=np.float32).reshape(1, D)

    inv_tau = 1.0 / tau                       # (1, D)
    a = 1.0 - inv_tau
    wstar = (weight * inv_tau).astype(np.float32)          # scale columns
    winstar = (input_weight * inv_tau).astype(np.float32)
    bstar = (bias * inv_tau).astype(np.float32)
    # per-partition layout (P, DB): col j holds elems [j*P, (j+1)*P)
    bstar_t = bstar.reshape(DB, P).T
    a_t = a.reshape(DB, P).T
    ident = np.eye(P, dtype=np.float32)

    cr = np.concatenate(
        [wstar[0:P, :], wstar[P:D, :],
         winstar[0:P, :], winstar[P:D, :], ident], axis=1)
    cf = np.concatenate([bstar_t, a_t, np.zeros((P, P), np.float32)], axis=1)

    shared = {
        "constsr": np.ascontiguousarray(cr),
        "constsf": np.ascontiguousarray(cf),
    }
    return [
        {"xt": np.ascontiguousarray(x[b].T), **shared} for b in range(NCORES)
    ]


def kernel(x, weight, input_weight, bias, tau, ksize, _trace=False):
    assert int(ksize) == KSIZE
    nc = get_program()
    in_maps = make_in_maps(x, weight, input_weight, bias, tau)
    res = run_bass_kernel_spmd(
        nc, in_maps, core_ids=list(range(NCORES)), trace=_trace
    )
    out = np.stack(
        [np.ascontiguousarray(res.results[b]["out"].T) for b in range(NCORES)],
        axis=0,
    )
    if _trace:
        _cache["last_results"] = res
    return out.astype(np.float32)


# revision 10
# speedup vs baseline: 1.2277x; 1.0328x over previous
"""Trainium2 Bass kernel for nn_LocalRNN: 8-step CTRNN over sliding windows.

Math:
  For each position l: h_{k+1} = a*h_k + relu(h_k @ W* + u*[l+k]),  h_0 = 0
  where a = 1 - 1/tau, W* = W * (1/tau) (columns), u* = Xp @ W_in* + b*,
  W_in* = W_in * (1/tau), b* = b * (1/tau).  Output = h_8 per position.
  (Uses relu(c*z) = c*relu(z) for c>0 to fold 1/tau into the weights, and
  the fact that the input projection is shared across overlapping windows.)

Sharding: batch dim (8) across the 8 NeuronCores, weights replicated.
On-chip layout is transposed ([d on partitions, positions on free dim]) so
matmuls contract d on the partition axis; the host uploads x pre-transposed
and transposes the [d, pos] output back (layout marshalling only).
"""

import numpy as np
from contextlib import ExitStack

import concourse.bass as bass
import concourse.tile as tile
from concourse import bacc, mybir
from concourse.bass_utils import run_bass_kernel_spmd

B, L, D, KSIZE = 8, 2048, 256, 8
P = 128
NCORES = 8
MMN = 512                    # matmul moving free dim (PSUM bank limit)
WCH = 1024                   # wide chunk for ACT/DVE elementwise ops
NW = L // WCH                # 2
NG = L // MMN                # 4 groups of 512
UCOLS = L + KSIZE - 1        # 2055
PAD = KSIZE - 1              # 7
DB = D // P                  # 2 d-blocks
F32 = mybir.dt.float32
F32R = mybir.dt.float32r
AF = mybir.ActivationFunctionType
ALU = mybir.AluOpType

# packed f32r consts blobs: cru = wint0|wint1 ; crw = wt0|wt1|identr
CRU_COLS = 2 * D
CRW_COLS = 2 * D + P
CRW_ID = 2 * D
# packed f32 consts blob: bst|at|pad src
CF_COLS = 2 * DB + P
_cache = {}


def _build_program():
    nc = bacc.Bacc(
        "TRN2",
        target_bir_lowering=False,
        debug=False,
        enable_asserts=False,
        num_devices=NCORES,
    )
    # x uploaded pre-transposed: (D, L), row d -> [d, positions]
    x_d = nc.dram_tensor("xt", (D, L), F32R, kind="ExternalInput").ap()
    cru_d = nc.dram_tensor("constsru", (P, CRU_COLS), F32R, kind="ExternalInput").ap()
    crw_d = nc.dram_tensor("constsrw", (P, CRW_COLS), F32R, kind="ExternalInput").ap()
    cf_d = nc.dram_tensor("constsf", (P, CF_COLS), F32, kind="ExternalInput").ap()
    # output in T-layout: (D, L); host transposes back
    out_d = nc.dram_tensor("out", (D, L), F32R, kind="ExternalOutput").ap()

    with tile.TileContext(nc) as tc, ExitStack() as ctx:
        consts = ctx.enter_context(tc.tile_pool(name="consts", bufs=1))
        big = ctx.enter_context(tc.tile_pool(name="big", bufs=1))
        rp = ctx.enter_context(tc.tile_pool(name="rp", bufs=3))
        # single PSUM pool, all tags share slots: [128,1024] slot = 2 banks,
        # bufs=4 -> 8 banks
        zp = ctx.enter_context(tc.tile_pool(name="zp", bufs=4, space="PSUM"))

        # --- PE warmup: dummy matmuls on garbage data to engage HAM early ---
        dummy_f = big.tile([P, MMN], F32, name="dummy_f")
        dummy = big.tile([P, MMN], F32R, name="dummy")
        nc.vector.memset(dummy_f[:], 0.0)
        nc.vector.tensor_copy(dummy[:], dummy_f[:])
        warm = zp.tile([P, MMN], F32, name="warm", tag="z")
        for _ in range(6):
            nc.tensor.matmul(warm[:], lhsT=dummy[:, 0:P], rhs=dummy[:],
                             start=True, stop=True)

        # --- constants ---
        cru = consts.tile([P, CRU_COLS], F32R, name="cru")
        crw = consts.tile([P, CRW_COLS], F32R, name="crw")
        cf = consts.tile([P, CF_COLS], F32, name="cf")
        wt = [crw[:, i * D:(i + 1) * D] for i in range(DB)]
        wint = [cru[:, i * D:(i + 1) * D] for i in range(DB)]
        identr = crw[:, CRW_ID:CRW_ID + P]
        bst = cf[:, 0:DB]
        at = cf[:, DB:2 * DB]

        # --- persistent buffers: d-blocks side by side in one tile ---
        xt_all = big.tile([P, DB * L], F32R, name="xt_all")
        xt = [xt_all[:, i * L:(i + 1) * L] for i in range(DB)]
        ut = [big.tile([P, UCOLS], F32R, name=f"ut{i}") for i in range(DB)]
        hball = [big.tile([P, DB * L], F32R, name=f"hb{s}") for s in range(2)]
        hb = [[hball[s][:, i * L:(i + 1) * L] for i in range(DB)]
              for s in range(2)]
        h1 = hb[1]

        # --- input: 4 x DMAs + 3 const DMAs, interleaved on both HWDGE
        # engines so the first u matmul's inputs (x g0, g1, wint) land first
        def xdma(eng, g):
            eng.dma_start(
                xt_all[:].rearrange("p (i c) -> p i c", i=DB)[
                    :, :, g * MMN:(g + 1) * MMN],
                x_d.rearrange("(i p) c -> p i c", p=P)[
                    :, :, g * MMN:(g + 1) * MMN],
            )
        xdma(nc.sync, 0)
        xdma(nc.scalar, 1)
        nc.sync.dma_start(cru[:], cru_d[:, :])
        nc.scalar.dma_start(cf[:], cf_d[:, :])
        xdma(nc.sync, 2)
        xdma(nc.scalar, 3)
        nc.sync.dma_start(crw[:], crw_d[:, :])

        # u pad cols + h1 pad cols (also warms the ACT table early):
        # u[:, :7] = b*, h1[:, :7] = relu(b*)
        for j in range(DB):
            nc.scalar.activation(
                ut[j][:, 0:PAD], cf[:, 0:PAD],
                AF.Identity, bias=bst[:, j:j + 1], scale=0.0,
            )
            nc.scalar.activation(
                h1[j][:, 0:PAD], cf[:, 0:PAD],
                AF.Relu, bias=bst[:, j:j + 1], scale=0.0,
            )


        # --- u projection, wide tiles; h1 (ACT) and u (DVE) read PSUM ---
        for gw in range(2):
            for j in range(DB):
                zt = zp.tile([P, WCH], F32, name="zu", tag="z")
                for half in range(2):
                    g = 2 * gw + half
                    zh = zt[:, half * MMN:(half + 1) * MMN]
                    for i in range(DB):
                        nc.tensor.matmul(
                            zh,
                            lhsT=wint[i][:, j * P:(j + 1) * P],
                            rhs=xt[i][:, g * MMN:(g + 1) * MMN],
                            start=(i == 0),
                            stop=(i == DB - 1),
                        )
                # h1 positions [7+1024gw, min(7+1024(gw+1), 2048))
                hw = WCH if gw == 0 else WCH - PAD
                nc.scalar.activation(
                    h1[j][:, PAD + gw * WCH:PAD + gw * WCH + hw],
                    zt[:, 0:hw], AF.Relu, bias=bst[:, j:j + 1], scale=1.0,
                )
                nc.vector.tensor_scalar(
                    out=ut[j][:, PAD + gw * WCH:PAD + (gw + 1) * WCH],
                    in0=zt[:],
                    scalar1=bst[:, j:j + 1],
                    scalar2=None,
                    op0=ALU.add,
                )

        # --- steps 1..6 (wide 1024-col chunks; matmuls in 512 halves) ---
        for k in range(1, KSIZE - 1):
            hc = hb[k % 2]
            hn = hb[(k + 1) % 2]
            for c in range(NW):
                cs = c * WCH
                for j in range(DB):
                    zt = zp.tile([P, WCH], F32, name="zt", tag="z")
                    for h in range(2):
                        hs = cs + h * MMN
                        zh = zt[:, h * MMN:(h + 1) * MMN]
                        for i in range(DB):
                            nc.tensor.matmul(
                                zh,
                                lhsT=wt[i][:, j * P:(j + 1) * P],
                                rhs=hc[i][:, hs:hs + MMN],
                                start=(i == 0),
                                stop=False,
                            )
                        nc.tensor.matmul(
                            zh,
                            lhsT=identr,
                            rhs=ut[j][:, k + hs:k + hs + MMN],
                            start=False,
                            stop=True,
                        )
                    r = rp.tile([P, WCH], F32, name="r", tag="r")
                    nc.scalar.activation(r[:], zt[:], AF.Relu)
                    nc.vector.scalar_tensor_tensor(
                        out=hn[j][:, cs:cs + WCH],
                        in0=hc[j][:, cs:cs + WCH],
                        scalar=at[:, j:j + 1],
                        in1=r[:],
                        op0=ALU.mult,
                        op1=ALU.add,
                    )

        # --- step 7 in 512-col chunks, output DMA per chunk ---
        k = KSIZE - 1
        hc = hb[k % 2]
        hn = hb[(k + 1) % 2]
        h8all = hball[(k + 1) % 2]
        for g in range(NG):
            cs = g * MMN
            for j in range(DB):
                zt = zp.tile([P, MMN], F32, name="z7", tag="z")
                for i in range(DB):
                    nc.tensor.matmul(
                        zt[:],
                        lhsT=wt[i][:, j * P:(j + 1) * P],
                        rhs=hc[i][:, cs:cs + MMN],
                        start=(i == 0),
                        stop=False,
                    )
                nc.tensor.matmul(
                    zt[:],
                    lhsT=identr,
                    rhs=ut[j][:, k + cs:k + cs + MMN],
                    start=False,
                    stop=True,
                )
                r = rp.tile([P, MMN], F32, name="r7", tag="r")
                nc.scalar.activation(r[:], zt[:], AF.Relu)
                nc.vector.scalar_tensor_tensor(
                    out=hn[j][:, cs:cs + MMN],
                    in0=hc[j][:, cs:cs + MMN],
                    scalar=at[:, j:j + 1],
                    in1=r[:],
                    op0=ALU.mult,
                    op1=ALU.add,
                )
            eng = nc.sync if g % 2 == 0 else nc.scalar
            eng.dma_start(
                out_d.rearrange("(i p) c -> p i c", p=P)[
                    :, :, cs:cs + MMN],
                h8all[:].rearrange("p (i c) -> p i c", i=DB)[
                    :, :, cs:cs + MMN],
            )

    nc.compile()
    return nc


def get_program():
    if "nc" not in _cache:
        _cache["nc"] = _build_program()
    return _cache["nc"]


def make_in_maps(x, weight, input_weight, bias, tau):
    x = np.asarray(x, dtype=np.float32)
    weight = np.asarray(weight, dtype=np.float32)
    input_weight = np.asarray(input_weight, dtype=np.float32)
    bias = np.asarray(bias, dtype=np.float32).reshape(1, D)
    tau = np.asarray(tau, dtype=np.float32).reshape(1, D)

    inv_tau = 1.0 / tau                       # (1, D)
    a = 1.0 - inv_tau
    wstar = (weight * inv_tau).astype(np.float32)          # scale columns
    winstar = (input_weight * inv_tau).astype(np.float32)
    bstar = (bias * inv_tau).astype(np.float32)
    # per-partition layout (P, DB): col j holds elems [j*P, (j+1)*P)
    bstar_t = bstar.reshape(DB, P).T
    a_t = a.reshape(DB, P).T
    ident = np.eye(P, dtype=np.float32)

    cru = np.concatenate([winstar[0:P, :], winstar[P:D, :]], axis=1)
    crw = np.concatenate([wstar[0:P, :], wstar[P:D, :], ident], axis=1)
    cf = np.concatenate([bstar_t, a_t, np.zeros((P, P), np.float32)], axis=1)

    shared = {
        "constsru": np.ascontiguousarray(cru),
        "constsrw": np.ascontiguousarray(crw),
        "constsf": np.ascontiguousarray(cf),
    }
    return [
        {"xt": np.ascontiguousarray(x[b].T), **shared} for b in range(NCORES)
    ]


def kernel(x, weight, input_weight, bias, tau, ksize, _trace=False):
    assert int(ksize) == KSIZE
    nc = get_program()
    in_maps = make_in_maps(x, weight, input_weight, bias, tau)
    res = run_bass_kernel_spmd(
        nc, in_maps, core_ids=list(range(NCORES)), trace=_trace
    )
    out = np.stack(
        [np.ascontiguousarray(res.results[b]["out"].T) for b in range(NCORES)],
        axis=0,
    )
    if _trace:
        _cache["last_results"] = res
    return out.astype(np.float32)


# revision 11
# speedup vs baseline: 1.2393x; 1.0094x over previous
"""Trainium2 Bass kernel for nn_LocalRNN: 8-step CTRNN over sliding windows.

Math:
  For each position l: h_{k+1} = a*h_k + relu(h_k @ W* + u*[l+k]),  h_0 = 0
  where a = 1 - 1/tau, W* = W * (1/tau) (columns), u* = Xp @ W_in* + b*,
  W_in* = W_in * (1/tau), b* = b * (1/tau).  Output = h_8 per position.
  (Uses relu(c*z) = c*relu(z) for c>0 to fold 1/tau into the weights, and
  the fact that the input projection is shared across overlapping windows.)

Sharding: batch dim (8) across the 8 NeuronCores, weights replicated.
On-chip layout is transposed ([d on partitions, positions on free dim]) so
matmuls contract d on the partition axis; the host uploads x pre-transposed
and transposes the [d, pos] output back (layout marshalling only).
"""

import numpy as np
from contextlib import ExitStack

import concourse.bass as bass
import concourse.tile as tile
from concourse import bacc, mybir
from concourse.bass_utils import run_bass_kernel_spmd

B, L, D, KSIZE = 8, 2048, 256, 8
P = 128
NCORES = 8
MMN = 512                    # matmul moving free dim (PSUM bank limit)
WCH = 1024                   # wide chunk for ACT/DVE elementwise ops
NW = L // WCH                # 2
NG = L // MMN                # 4 groups of 512
UCOLS = L + KSIZE - 1        # 2055
PAD = KSIZE - 1              # 7
DB = D // P                  # 2 d-blocks
F32 = mybir.dt.float32
F32R = mybir.dt.float32r
AF = mybir.ActivationFunctionType
ALU = mybir.AluOpType

# packed f32r consts blobs: cru = wint0|wint1 ; crw = wt0|wt1|identr
CRU_COLS = 2 * D
CRW_COLS = 2 * D + P
CRW_ID = 2 * D
# packed f32 consts blob: bst|at|pad src
CF_COLS = 2 * DB + P
_cache = {}


def _build_program():
    nc = bacc.Bacc(
        "TRN2",
        target_bir_lowering=False,
        debug=False,
        enable_asserts=False,
        num_devices=NCORES,
    )
    # x uploaded pre-transposed: (D, L), row d -> [d, positions]
    x_d = nc.dram_tensor("xt", (D, L), F32R, kind="ExternalInput").ap()
    cru_d = nc.dram_tensor("constsru", (P, CRU_COLS), F32R, kind="ExternalInput").ap()
    crw_d = nc.dram_tensor("constsrw", (P, CRW_COLS), F32R, kind="ExternalInput").ap()
    cf_d = nc.dram_tensor("constsf", (P, CF_COLS), F32, kind="ExternalInput").ap()
    # output in T-layout: (D, L); host transposes back
    out_d = nc.dram_tensor("out", (D, L), F32R, kind="ExternalOutput").ap()

    with tile.TileContext(nc) as tc, ExitStack() as ctx:
        consts = ctx.enter_context(tc.tile_pool(name="consts", bufs=1))
        big = ctx.enter_context(tc.tile_pool(name="big", bufs=1))
        rp = ctx.enter_context(tc.tile_pool(name="rp", bufs=3))
        # single PSUM pool, all tags share slots: [128,1024] slot = 2 banks,
        # bufs=4 -> 8 banks
        zp = ctx.enter_context(tc.tile_pool(name="zp", bufs=4, space="PSUM"))

        # --- PE warmup: dummy matmuls on garbage data to engage HAM early ---
        dummy_f = big.tile([P, MMN], F32, name="dummy_f")
        dummy = big.tile([P, MMN], F32R, name="dummy")
        nc.vector.memset(dummy_f[:], 0.0)
        nc.vector.tensor_copy(dummy[:], dummy_f[:])
        warm = zp.tile([P, MMN], F32, name="warm", tag="z")
        for _ in range(8):
            nc.tensor.matmul(warm[:], lhsT=dummy[:, 0:P], rhs=dummy[:],
                             start=True, stop=True)

        # --- constants ---
        cru = consts.tile([P, CRU_COLS], F32R, name="cru")
        crw = consts.tile([P, CRW_COLS], F32R, name="crw")
        cf = consts.tile([P, CF_COLS], F32, name="cf")
        wt = [crw[:, i * D:(i + 1) * D] for i in range(DB)]
        wint = [cru[:, i * D:(i + 1) * D] for i in range(DB)]
        identr = crw[:, CRW_ID:CRW_ID + P]
        bst = cf[:, 0:DB]
        at = cf[:, DB:2 * DB]

        # --- persistent buffers ---
        # x in 4 per-chunk tiles so each u matmul waits only on its own DMA
        xtg = [big.tile([P, DB * MMN], F32R, name=f"xtg{g}") for g in range(NG)]
        ut = [big.tile([P, UCOLS], F32R, name=f"ut{i}") for i in range(DB)]
        hball = [big.tile([P, DB * L], F32R, name=f"hb{s}") for s in range(2)]
        hb = [[hball[s][:, i * L:(i + 1) * L] for i in range(DB)]
              for s in range(2)]
        h1 = hb[1]

        # --- input: 4 x DMAs + 3 const DMAs, interleaved on both HWDGE
        # engines so the first u matmul's inputs (x g0, g1, wint) land first
        def xdma(eng, g):
            eng.dma_start(
                xtg[g][:].rearrange("p (i c) -> p i c", i=DB),
                x_d.rearrange("(i p) c -> p i c", p=P)[
                    :, :, g * MMN:(g + 1) * MMN],
            )
        xdma(nc.sync, 0)
        xdma(nc.scalar, 1)
        nc.sync.dma_start(cru[:], cru_d[:, :])
        nc.scalar.dma_start(cf[:], cf_d[:, :])
        xdma(nc.sync, 2)
        xdma(nc.scalar, 3)
        nc.sync.dma_start(crw[:], crw_d[:, :])

        # u pad cols + h1 pad cols (also warms the ACT table early):
        # u[:, :7] = b*, h1[:, :7] = relu(b*)
        for j in range(DB):
            nc.scalar.activation(
                ut[j][:, 0:PAD], cf[:, 0:PAD],
                AF.Identity, bias=bst[:, j:j + 1], scale=0.0,
            )
            nc.scalar.activation(
                h1[j][:, 0:PAD], cf[:, 0:PAD],
                AF.Relu, bias=bst[:, j:j + 1], scale=0.0,
            )


        # --- u projection, wide tiles; h1 (ACT) and u (DVE) read PSUM ---
        for gw in range(2):
            for j in range(DB):
                zt = zp.tile([P, WCH], F32, name="zu", tag="z")
                for half in range(2):
                    g = 2 * gw + half
                    zh = zt[:, half * MMN:(half + 1) * MMN]
                    for i in range(DB):
                        nc.tensor.matmul(
                            zh,
                            lhsT=wint[i][:, j * P:(j + 1) * P],
                            rhs=xtg[g][:, i * MMN:(i + 1) * MMN],
                            start=(i == 0),
                            stop=(i == DB - 1),
                        )
                # h1 positions [7+1024gw, min(7+1024(gw+1), 2048))
                hw = WCH if gw == 0 else WCH - PAD
                nc.scalar.activation(
                    h1[j][:, PAD + gw * WCH:PAD + gw * WCH + hw],
                    zt[:, 0:hw], AF.Relu, bias=bst[:, j:j + 1], scale=1.0,
                )
                nc.vector.tensor_scalar(
                    out=ut[j][:, PAD + gw * WCH:PAD + (gw + 1) * WCH],
                    in0=zt[:],
                    scalar1=bst[:, j:j + 1],
                    scalar2=None,
                    op0=ALU.add,
                )

        # --- steps 1..6 (wide 1024-col chunks; matmuls in 512 halves) ---
        for k in range(1, KSIZE - 1):
            hc = hb[k % 2]
            hn = hb[(k + 1) % 2]
            for c in range(NW):
                cs = c * WCH
                for j in range(DB):
                    zt = zp.tile([P, WCH], F32, name="zt", tag="z")
                    for h in range(2):
                        hs = cs + h * MMN
                        zh = zt[:, h * MMN:(h + 1) * MMN]
                        for i in range(DB):
                            nc.tensor.matmul(
                                zh,
                                lhsT=wt[i][:, j * P:(j + 1) * P],
                                rhs=hc[i][:, hs:hs + MMN],
                                start=(i == 0),
                                stop=False,
                            )
                        nc.tensor.matmul(
                            zh,
                            lhsT=identr,
                            rhs=ut[j][:, k + hs:k + hs + MMN],
                            start=False,
                            stop=True,
                        )
                    r = rp.tile([P, WCH], F32, name="r", tag="r")
                    nc.scalar.activation(r[:], zt[:], AF.Relu)
                    nc.vector.scalar_tensor_tensor(
                        out=hn[j][:, cs:cs + WCH],
                        in0=hc[j][:, cs:cs + WCH],
                        scalar=at[:, j:j + 1],
                        in1=r[:],
                        op0=ALU.mult,
                        op1=ALU.add,
                    )

        # --- step 7 in 512-col chunks, output DMA per chunk ---
        k = KSIZE - 1
        hc = hb[k % 2]
        hn = hb[(k + 1) % 2]
        h8all = hball[(k + 1) % 2]
        for g in range(NG):
            cs = g * MMN
            for j in range(DB):
                zt = zp.tile([P, MMN], F32, name="z7", tag="z")
                for i in range(DB):
                    nc.tensor.matmul(
                        zt[:],
                        lhsT=wt[i][:, j * P:(j + 1) * P],
                        rhs=hc[i][:, cs:cs + MMN],
                        start=(i == 0),
                        stop=False,
                    )
                nc.tensor.matmul(
                    zt[:],
                    lhsT=identr,
                    rhs=ut[j][:, k + cs:k + cs + MMN],
                    start=False,
                    stop=True,
                )
                r = rp.tile([P, MMN], F32, name="r7", tag="r")
                nc.scalar.activation(r[:], zt[:], AF.Relu)
                nc.vector.scalar_tensor_tensor(
                    out=hn[j][:, cs:cs + MMN],
                    in0=hc[j][:, cs:cs + MMN],
                    scalar=at[:, j:j + 1],
                    in1=r[:],
                    op0=ALU.mult,
                    op1=ALU.add,
                )
            eng = nc.sync if g % 2 == 0 else nc.scalar
            eng.dma_start(
                out_d.rearrange("(i p) c -> p i c", p=P)[
                    :, :, cs:cs + MMN],
                h8all[:].rearrange("p (i c) -> p i c", i=DB)[
                    :, :, cs:cs + MMN],
            )

    nc.compile()
    return nc


def get_program():
    if "nc" not in _cache:
        _cache["nc"] = _build_program()
    return _cache["nc"]


def make_in_maps(x, weight, input_weight, bias, tau):
    x = np.asarray(x, dtype=np.float32)
    weight = np.asarray(weight, dtype=np.float32)
    input_weight = np.asarray(input_weight, dtype=np.float32)
    bias = np.asarray(bias, dtype=np.float32).reshape(1, D)
    tau = np.asarray(tau, dtype=np.float32).reshape(1, D)

    inv_tau = 1.0 / tau                       # (1, D)
    a = 1.0 - inv_tau
    wstar = (weight * inv_tau).astype(np.float32)          # scale columns
    winstar = (input_weight * inv_tau).astype(np.float32)
    bstar = (bias * inv_tau).astype(np.float32)
    # per-partition layout (P, DB): col j holds elems [j*P, (j+1)*P)
    bstar_t = bstar.reshape(DB, P).T
    a_t = a.reshape(DB, P).T
    ident = np.eye(P, dtype=np.float32)

    cru = np.concatenate([winstar[0:P, :], winstar[P:D, :]], axis=1)
    crw = np.concatenate([wstar[0:P, :], wstar[P:D, :], ident], axis=1)
    cf = np.concatenate([bstar_t, a_t, np.zeros((P, P), np.float32)], axis=1)

    shared = {
        "constsru": np.ascontiguousarray(cru),
        "constsrw": np.ascontiguousarray(crw),
        "constsf": np.ascontiguousarray(cf),
    }
    return [
        {"xt": np.ascontiguousarray(x[b].T), **shared} for b in range(NCORES)
    ]


def kernel(x, weight, input_weight, bias, tau, ksize, _trace=False):
    assert int(ksize) == KSIZE
    nc = get_program()
    in_maps = make_in_maps(x, weight, input_weight, bias, tau)
    res = run_bass_kernel_spmd(
        nc, in_maps, core_ids=list(range(NCORES)), trace=_trace
    )
    out = np.stack(
        [np.ascontiguousarray(res.results[b]["out"].T) for b in range(NCORES)],
        axis=0,
    )
    if _trace:
        _cache["last_results"] = res
    return out.astype(np.float32)
